# revision 2
# baseline (speedup 1.0000x reference)
"""HadLinear Trainium2 kernel v4: per-128-block L2-norm -> Hadamard -> 4-bit
Gaussian-codebook quantize -> rescale -> matmul with w.T/sqrt(128).

Sharding: 8-way data-parallel over tokens (16384 tokens / 8 cores = 2048 each).
Weight host-pre-transposed to [in_dim, out_dim] fp16, replicated per core.

v4 design changes over v3:
 - Quantize staircase runs in the SQUARED domain: ACT Square(hp) serves both
   the norm reduction (cs = ones @ hp^2 = 128*s^2, Parseval) and the compares
   (u^2 vs b_i^2) -- the x^2 pass and the Abs pass are gone.
 - 1/(128 s^2) via nc.vector.reciprocal_approx_fast (fp32, ~5x faster than
   the iterative-divide stock reciprocal).
 - PE emission interleaves each next-group quantize macro (hadamard + norm
   matmuls) between the main-matmul n-chunks so the FIFO PE queue never
   stalls on the ACT round-trip (hp -> Square -> cs).
 - ACT uses only {Square, Sign, Sqrt, Copy} = one activation table set.
"""

import math

import numpy as np

# ---------------------------------------------------------------- constants
BS = 128          # hadamard block size
NLEVELS = 16

_consts = None


def _get_consts():
    global _consts
    if _consts is not None:
        return _consts
    import jax

    _p = (np.arange(NLEVELS) + 0.5) / NLEVELS
    # mirror reference.py exactly (fp32 jax ppf)
    cent = np.asarray(jax.scipy.stats.norm.ppf(_p), dtype=np.float32)
    bound = np.asarray(
        (np.asarray(cent[1:]) + np.asarray(cent[:-1])) * np.float32(0.5),
        dtype=np.float32,
    )
    cpos = cent[8:16].copy()
    bpos = bound[8:15].copy()
    dpos = (cpos[1:] - cpos[:-1]).astype(np.float32)  # 7 deltas
    _consts = (cent, bound, cpos, bpos, dpos)
    return _consts


def _hadamard_matrix():
    x = np.eye(BS, dtype=np.float32)
    h = 1
    while h < BS:
        x = x.reshape(BS, -1, 2, h)
        a, b = x[:, :, 0, :], x[:, :, 1, :]
        x = np.concatenate([a + b, a - b], axis=-1)
        h *= 2
    return np.ascontiguousarray(x.reshape(BS, BS))


# ---------------------------------------------------------------- builder
def build_module(tok, d, num_devices=8, bpm=4):
    """Build the per-core bass program.

    tok: tokens per core; d: feature dim; bpm: 128-blocks per macro-tile.
    """
    import concourse.bass as bass
    import concourse.tile as tile
    from concourse import bacc, mybir

    f32 = mybir.dt.float32
    f16 = mybir.dt.float16
    A = mybir.AluOpType
    AF = mybir.ActivationFunctionType

    nb = d // BS                    # 128-blocks per row (32)
    nchunk_n = 512
    nn = d // nchunk_n              # output-col chunks (8)
    nmac = nb // bpm                # quantize macro tiles per group (8)
    # token group schedule: geometric ramp so quantize(g+1) (~0.29us/tok on
    # DVE) always finishes inside matmul(g) (~0.43us/tok on PE)
    if tok == 2048:
        groups = [128, 256, 384, 512, 512, 256]
    else:
        groups = [128, 384] + [512] * ((tok - 512) // 512)
    assert sum(groups) == tok
    gstart = list(np.cumsum([0] + groups[:-1]))
    ngroup = len(groups)

    _, _, cpos, bpos, dpos = _get_consts()
    # squared-domain boundaries (1/128 folds the hadamard scale: u2 compares
    # run on hp^2/(128 s^2) = (xh/sqrt(128))^2); final scale via sqrt scale
    bpos2 = [float(np.float32(b) * np.float32(b) / np.float32(128.0))
             for b in bpos]
    sb_scale = float(1.0 / (128.0 * 128.0 * 128.0))  # cs=128 s^2 -> s/128

    nc = bacc.Bacc(
        "TRN2", target_bir_lowering=False, debug=False,
        num_devices=num_devices,
    )
    xt_in = nc.dram_tensor("xt_in", [d, tok], f16, kind="ExternalInput").ap()
    w_t = nc.dram_tensor("w_t", [d, d], f16, kind="ExternalInput").ap()
    hmat_d = nc.dram_tensor("hmat", [BS, BS], f16, kind="ExternalInput").ap()
    out = nc.dram_tensor("out", [tok, d], f16, kind="ExternalOutput").ap()

    xt_v = xt_in.rearrange("(b p) t -> p b t", p=BS)  # [128, nb, tok]
    wt_v = w_t.rearrange("(k p) n -> p k n", p=BS)    # [128, nb, d]

    with tile.TileContext(nc) as tc:
        import contextlib

        ctx = contextlib.ExitStack()
        with ctx:
            singles = ctx.enter_context(tc.tile_pool(name="singles", bufs=1))
            xtc_p = ctx.enter_context(tc.tile_pool(name="xtc", bufs=3))
            sqh_p = ctx.enter_context(tc.tile_pool(name="sqh", bufs=2))
            sgn_p = ctx.enter_context(tc.tile_pool(name="sgn", bufs=2))
            sbt_p = ctx.enter_context(tc.tile_pool(name="sbt", bufs=2))
            rc_p = ctx.enter_context(tc.tile_pool(name="rc", bufs=2))
            acc_p = ctx.enter_context(tc.tile_pool(name="acc", bufs=2))
            mk_p = ctx.enter_context(tc.tile_pool(name="mk", bufs=2))
            xqg_p = ctx.enter_context(tc.tile_pool(name="xqg", bufs=2))
            w_p = ctx.enter_context(tc.tile_pool(name="wsl", bufs=2))
            ev_p = ctx.enter_context(tc.tile_pool(name="ev", bufs=3))
            # hp and cs share one 4-deep PSUM pool (hp_j freed by the ACT
            # square/sign reads before cs_j allocates) -> 4+2 banks of 8.
            hcs_p = ctx.enter_context(
                tc.tile_pool(name="hcs", bufs=4, space="PSUM"))
            mp_p = ctx.enter_context(
                tc.tile_pool(name="mp", bufs=3, space="PSUM"))

            hmat_s = singles.tile([BS, BS], f16)
            nc.sync.dma_start(out=hmat_s[:], in_=hmat_d[:, :])
            ones_s = singles.tile([BS, BS], f16)
            nc.vector.memset(ones_s[:], 1.0)

            state = {}

            def q_macro_start(g, mac):
                """Hadamard + squares for one macro (bpm blocks) of group g.
                Emits the PE hp matmuls and the ACT square/sign passes.
                Returns context for q_macro_finish."""
                t0, gtg = gstart[g], groups[g]
                b0 = mac * bpm
                mwg = bpm * gtg
                xtc = xtc_p.tile([BS, bpm, gtg], f16, tag="xtc")
                nc.sync.dma_start(
                    out=xtc[:], in_=xt_v[:, b0:b0 + bpm, t0:t0 + gtg])
                sqh = sqh_p.tile([BS, mwg], f16, tag="sqh")
                sgn = sgn_p.tile([BS, mwg], f16, tag="sgn")
                hps = []
                for j in range(bpm):
                    hp = hcs_p.tile([BS, gtg], f32, tag="hcs")
                    nc.tensor.matmul(hp[:], lhsT=hmat_s[:],
                                     rhs=xtc[:, j, :],
                                     start=True, stop=True)
                    hps.append(hp)
                for j in range(bpm):
                    sl = slice(j * gtg, (j + 1) * gtg)
                    nc.scalar.activation(out=sqh[:, sl], in_=hps[j][:],
                                         func=AF.Square)
                    nc.scalar.activation(out=sgn[:, sl], in_=hps[j][:],
                                         func=AF.Sign)
                return (g, mac, sqh, sgn)

            def q_macro_mid(qctx):
                """Norm matmuls (ones @ hp^2) for the macro."""
                g, mac, sqh, sgn = qctx
                t0, gtg = gstart[g], groups[g]
                css = []
                for j in range(bpm):
                    sl = slice(j * gtg, (j + 1) * gtg)
                    cs = hcs_p.tile([BS, gtg], f32, tag="hcs")
                    nc.tensor.matmul(cs[:], lhsT=ones_s[:],
                                     rhs=sqh[:, sl],
                                     start=True, stop=True)
                    css.append(cs)
                return (g, mac, sqh, sgn, css)

            def q_macro_finish(qctx, xqg):
                """Scales + staircase + final quantized writes (ACT+DVE)."""
                g, mac, sqh, sgn, css = qctx
                t0, gtg = gstart[g], groups[g]
                b0 = mac * bpm
                mwg = bpm * gtg
                sbt = sbt_p.tile([BS, mwg], f16, tag="sbt")
                for j in range(bpm):
                    sl = slice(j * gtg, (j + 1) * gtg)
                    cs = css[j]
                    # sbt = sqrt(cs/128^3) = s/128
                    nc.scalar.activation(out=sbt[:, sl], in_=cs[:],
                                         func=AF.Sqrt, scale=sb_scale)
                    rc = rc_p.tile([BS, gtg], f32, tag="rc")
                    nc.vector.reciprocal_approx_fast(out=rc[:], in_=cs[:])
                    # u2 = hp^2 / (128 s^2), in place over sqh
                    nc.vector.tensor_mul(sqh[:, sl], sqh[:, sl], rc[:])
                # staircase: acc = c0 + sum_i [u2 > b_i^2] * d_i
                acc = acc_p.tile([BS, mwg], f16, tag="acc")
                nc.vector.tensor_scalar(
                    out=acc[:], in0=sqh[:],
                    scalar1=bpos2[0], scalar2=float(dpos[0]),
                    op0=A.is_gt, op1=A.mult)
                for i in range(1, 7):
                    mk = mk_p.tile([BS, mwg], f16, tag="mk")
                    nc.vector.tensor_scalar(
                        out=mk[:], in0=sqh[:],
                        scalar1=bpos2[i], scalar2=float(dpos[i]),
                        op0=A.is_gt, op1=A.mult)
                    nc.vector.tensor_add(acc[:], acc[:], mk[:])
                nc.vector.tensor_scalar_add(acc[:], acc[:], float(cpos[0]))
                # ssu = sign * s/128 ; xq = staircase * ssu
                nc.vector.tensor_mul(sgn[:], sgn[:], sbt[:])
                xq_v = xqg[:, b0:b0 + bpm, :].rearrange("p b t -> p (b t)")
                nc.vector.tensor_mul(xq_v, acc[:], sgn[:])

            def mm_nchunk(g, xqg, n, part):
                """Main matmul for n-chunk n of group g; part in (0,1) emits
                half the m-tiles each."""
                t0, gtg = gstart[g], groups[g]
                tpg = gtg // 128
                if part == 0:
                    wsl = w_p.tile([BS, nb, nchunk_n], f16, tag="wsl")
                    state[("wsl", g, n)] = wsl
                    nc.sync.dma_start(
                        out=wsl[:],
                        in_=wt_v[:, :, n * nchunk_n:(n + 1) * nchunk_n])
                else:
                    wsl = state[("wsl", g, n)]
                mlo = (tpg * part) // 2
                mhi = (tpg * (part + 1)) // 2
                for m in range(mlo, mhi):
                    ps = mp_p.tile([BS, nchunk_n], f32, tag="mp")
                    for k in range(nb):
                        nc.tensor.matmul(
                            ps[:],
                            lhsT=xqg[:, k, m * 128:(m + 1) * 128],
                            rhs=wsl[:, k, :],
                            start=(k == 0), stop=(k == nb - 1))
                    ev = ev_p.tile([BS, nchunk_n], f16, tag="ev")
                    nc.scalar.copy(out=ev[:], in_=ps[:])
                    nc.sync.dma_start(
                        out=out[t0 + m * 128:t0 + (m + 1) * 128,
                                n * nchunk_n:(n + 1) * nchunk_n],
                        in_=ev[:])

            def quantize_group_solid(g):
                """Un-interleaved quantize of a whole group (pipeline head)."""
                xqg = xqg_p.tile([BS, nb, groups[g]], f16, tag="xqg")
                for mac in range(nmac):
                    qc = q_macro_start(g, mac)
                    qc = q_macro_mid(qc)
                    q_macro_finish(qc, xqg)
                return xqg

            # ---- emission schedule ----
            # head: quantize group 0 solid
            xq_cur = quantize_group_solid(0)
            for g in range(ngroup):
                if g + 1 < ngroup:
                    xq_nxt = xqg_p.tile(
                        [BS, nb, groups[g + 1]], f16, tag="xqg")
                    # interleave: 8 n-chunks of M(g) with 8 q-macros of g+1
                    for n in range(nn):
                        qc = q_macro_start(g + 1, n)
                        mm_nchunk(g, xq_cur, n, 0)
                        qc = q_macro_mid(qc)
                        mm_nchunk(g, xq_cur, n, 1)
                        q_macro_finish(qc, xq_nxt)
                    xq_cur = xq_nxt
                else:
                    for n in range(nn):
                        mm_nchunk(g, xq_cur, n, 0)
                        mm_nchunk(g, xq_cur, n, 1)

    nc.compile()
    return nc


# ---------------------------------------------------------------- driver
_CACHED = None

TOK_FULL = 2048
D_FULL = 4096


def _get_compiled():
    global _CACHED
    if _CACHED is None:
        from concourse.bass_interp import get_hw_module

        nc = build_module(TOK_FULL, D_FULL, num_devices=8)
        nc.m = get_hw_module(nc.m)
        _CACHED = nc
    return _CACHED


def _run(input, weight, trace=False):
    from concourse import bass_utils

    nc = _get_compiled()
    x = np.asarray(input, dtype=np.float32).reshape(-1, D_FULL)
    xt = np.ascontiguousarray(x.T.astype(np.float16))  # [d, tok_total]
    wt = np.ascontiguousarray(
        np.asarray(weight, dtype=np.float32).T).astype(np.float16)
    hm = _hadamard_matrix().astype(np.float16)
    ncores = 8
    in_maps = [
        {"xt_in": np.ascontiguousarray(
            xt[:, i * TOK_FULL:(i + 1) * TOK_FULL]),
         "w_t": wt, "hmat": hm}
        for i in range(ncores)
    ]
    res = bass_utils.run_bass_kernel_spmd(
        nc, in_maps, core_ids=list(range(ncores)), trace=trace)
    outs = [res.results[i]["out"] for i in range(ncores)]
    full = np.concatenate(outs, axis=0).astype(np.float32).reshape(input.shape)
    return full, res


def kernel(input, weight):
    out, _ = _run(input, weight, trace=False)
    return out
